# revision 1
# baseline (speedup 1.0000x reference)
import numpy as np

# nn_EAS4_46986942218814 — focused linear attention + MixFFN block.
# Shapes are fixed by the problem: x (8, 512, 64, 64) f32.
B, C, H, W = 8, 512, 64, 64
N = H * W
HID = 4 * C
FOCUS = 3.0


def _erf(x):
    try:
        from scipy.special import erf
        return erf(x).astype(np.float32)
    except Exception:
        # Abramowitz–Stegun 7.1.26 fallback (abs err < 1.5e-7)
        sign = np.sign(x)
        ax = np.abs(x)
        t = 1.0 / (1.0 + 0.3275911 * ax)
        y = 1.0 - (((((1.061405429 * t - 1.453152027) * t) + 1.421413741) * t
                    - 0.284496736) * t + 0.254829592) * t * np.exp(-ax * ax)
        return (sign * y).astype(np.float32)


def _ln(x, g, b, eps=1e-5):
    m = x.mean(-1, keepdims=True, dtype=np.float32)
    v = ((x - m) ** 2).mean(-1, keepdims=True, dtype=np.float32)
    return (x - m) / np.sqrt(v + eps) * g + b


def _softmax(x, axis):
    x = x - x.max(axis=axis, keepdims=True)
    e = np.exp(x)
    return e / e.sum(axis=axis, keepdims=True)


def kernel(x, Wq, bq, Wk, bk, Wv, bv, scale, fc1_w, fc1_b, dw_w, dw_b,
           fc2_w, fc2_b, ln1_g, ln1_b, lnm_g, lnm_b):
    x = np.asarray(x, np.float32)
    b = x.shape[0]
    xf = x.reshape(b, C, N)                      # (B, C, N)

    # 1x1 convs as matmuls over the channel dim, then RAW reshape to (B, N, C)
    def conv1(Wm, bm):
        out = np.einsum('oc,bcn->bon', np.asarray(Wm, np.float32), xf,
                        optimize=True) + np.asarray(bm, np.float32)[None, :, None]
        return out.reshape(b, N, C)              # raw reshape, no permute

    q = _softmax(conv1(Wq, bq), axis=1)
    k = _softmax(conv1(Wk, bk), axis=2)
    v = conv1(Wv, bv)

    q = np.maximum(q, 0.0) + 1e-6
    k = np.maximum(k, 0.0) + 1e-6
    sc = np.log1p(np.exp(np.asarray(scale, np.float32)))   # softplus
    q = q / sc
    k = k / sc

    q_norm = np.linalg.norm(q, axis=-1, keepdims=True)
    k_norm = np.linalg.norm(k, axis=-1, keepdims=True)
    q = q ** FOCUS
    k = k ** FOCUS
    q = q / np.linalg.norm(q, axis=-1, keepdims=True) * q_norm
    k = k / np.linalg.norm(k, axis=-1, keepdims=True) * k_norm

    z = 1.0 / (np.einsum('bic,bc->bi', q, k.sum(axis=1), optimize=True) + 1e-6)
    kv = np.einsum('bjc,bjd->bcd', k, v, optimize=True)
    attn = np.einsum('bic,bcd->bid', q, kv, optimize=True) * z[:, :, None]

    shortcut = np.transpose(x, (0, 2, 3, 1)).reshape(b, N, C)
    enhanced = shortcut + attn

    # MixFFN
    t = _ln(enhanced, np.asarray(lnm_g, np.float32), np.asarray(lnm_b, np.float32))
    a = t @ np.asarray(fc1_w, np.float32) + np.asarray(fc1_b, np.float32)  # (B,N,HID)

    # depthwise 3x3 SAME conv on (B, HID, H, W)
    ai = np.transpose(a, (0, 2, 1)).reshape(b, HID, H, W)
    pad = np.zeros((b, HID, H + 2, W + 2), np.float32)
    pad[:, :, 1:-1, 1:-1] = ai
    wdw = np.asarray(dw_w, np.float32)           # (HID, 1, 3, 3)
    dw = np.zeros((b, HID, H, W), np.float32)
    for dy in range(3):
        for dx in range(3):
            dw += pad[:, :, dy:dy + H, dx:dx + W] * wdw[None, :, 0, dy, dx, None, None]
    dw += np.asarray(dw_b, np.float32)[None, :, None, None]
    dw = np.transpose(dw.reshape(b, HID, N), (0, 2, 1))   # (B, N, HID)

    h1 = _ln(dw + a, np.asarray(ln1_g, np.float32), np.asarray(ln1_b, np.float32))
    ax = 0.5 * h1 * (1.0 + _erf(h1 / np.sqrt(np.float32(2.0))))  # exact gelu
    mlp_out = ax @ np.asarray(fc2_w, np.float32) + np.asarray(fc2_b, np.float32)
    out = enhanced + mlp_out
    return out.reshape(b, H, W, C).astype(np.float32)



# revision 8
# speedup vs baseline: 5.5876x; 5.5876x over previous
"""nn_EAS4_46986942218814 — focused linear attention + MixFFN, Bass/Tile TRN2 kernel.

Data-parallel over batch: 8 samples -> 8 NeuronCores, identical SPMD program.
All matmuls bf16 with f32 PSUM accumulation. Self-contained; shapes hardcoded.

Layout notes (per core, one sample):
  x (c, n) and xT (n, c) both host-provided in bf16.
  Yq/Yk/Yv computed as (o, s) tiles; the torch "raw reshape" to (N, C) means
  A[n2, j] = Y[o = n2//8, s = 512*(n2%8) + j], so each A-row is a contiguous
  512-slice of one o-partition: k's row-softmax / focus chain are free-dim ops.
  q needs column ops -> DRAM bounce into A-layout, PE ones-matmul column sums.
  kv/attn/fc1/fc2 all contract over the partition dim with bf16 matmuls.
  The 3x3 depthwise conv runs on the PE as 9 diagonal-lhsT matmuls with
  spatially shifted rhs access patterns accumulating in one PSUM bank.
  LayerNorm statistics over partition-dim layouts come from PE ones-matmuls.
"""
import numpy as np
import ml_dtypes

B, C, H, W = 8, 512, 64, 64
N = H * W            # 4096
HID = 4 * C          # 2048
NCORES = 8

_CACHE = {}


def _build(phases=99):
    import os
    import concourse.bass as bass
    import concourse.tile as tile
    from concourse import bacc, mybir

    F32 = mybir.dt.float32
    BF16 = mybir.dt.bfloat16
    AF = mybir.ActivationFunctionType
    MUL = mybir.AluOpType.mult
    ADD = mybir.AluOpType.add

    nc = bacc.Bacc("TRN2", target_bir_lowering=False, debug=False,
                   num_devices=NCORES)

    d_x = nc.dram_tensor("x", [C, N], BF16, kind="ExternalInput")
    d_xT = nc.dram_tensor("xT", [N, C], BF16, kind="ExternalInput")
    d_wqkvT = nc.dram_tensor("wqkvT", [C, 3 * C], BF16, kind="ExternalInput")
    d_bqkv = nc.dram_tensor("bqkv", [128, 4, 3], F32, kind="ExternalInput")
    d_invsc = nc.dram_tensor("invsc_bc", [128, C], BF16, kind="ExternalInput")
    d_fc1 = nc.dram_tensor("fc1p", [C, HID], BF16, kind="ExternalInput")
    d_fc1b = nc.dram_tensor("fc1bp", [128, 16], F32, kind="ExternalInput")
    d_fc2 = nc.dram_tensor("fc2w", [HID, C], BF16, kind="ExternalInput")
    d_fc2b = nc.dram_tensor("fc2b", [1, C], BF16, kind="ExternalInput")
    d_dwd = nc.dram_tensor("dwdiag", [16, 128, 9 * 128], BF16, kind="ExternalInput")
    d_dwb = nc.dram_tensor("dwb", [128, 16], F32, kind="ExternalInput")
    d_ln1g = nc.dram_tensor("ln1g", [128, 16], F32, kind="ExternalInput")
    d_ln1b = nc.dram_tensor("ln1b", [128, 16], F32, kind="ExternalInput")
    d_eye = nc.dram_tensor("eye", [128, 128], BF16, kind="ExternalInput")
    d_onesc = nc.dram_tensor("onesc", [128, 1], BF16, kind="ExternalInput")
    d_onesr = nc.dram_tensor("onesr", [1, 128], BF16, kind="ExternalInput")
    d_out = nc.dram_tensor("out", [N, C], BF16, kind="ExternalOutput")
    d_eqs = nc.dram_tensor("eq_scr", [C, N], BF16)
    d_kss = nc.dram_tensor("ksum_scr", [1, C], BF16)
    d_ascr = nc.dram_tensor("a_scr", [16, 128, N], BF16)
    d_yscr = nc.dram_tensor("y_scr", [16, 128, N], BF16)

    class _Done(Exception):
        pass

    with tile.TileContext(nc) as tc:
        with (
            tc.tile_pool(name="cst", bufs=1) as cst,
            tc.tile_pool(name="big", bufs=1) as big,
            tc.tile_pool(name="strm", bufs=2) as strm,
            tc.tile_pool(name="chn", bufs=3) as chn,
            tc.tile_pool(name="vecp", bufs=8) as vecp,
            tc.tile_pool(name="ps", bufs=1, space="PSUM") as ps,
        ):
          try:
            def CT(shape, dt, tg):
                return cst.tile(shape, dt, tag=tg, name=tg)

            def V():
                return vecp.tile([1, 512], F32, tag="v", bufs=6, name="v")

            def Vb():
                return vecp.tile([1, 512], BF16, tag="vb", bufs=2, name="vb")

            # ---- constants ----
            t_eye = CT([128, 128], BF16, "eye"); nc.sync.dma_start(t_eye[:], d_eye[:])
            t_oc = CT([128, 1], BF16, "oc"); nc.sync.dma_start(t_oc[:], d_onesc[:])
            t_or = CT([1, 128], BF16, "onr"); nc.sync.dma_start(t_or[:], d_onesr[:])
            t_bqkv = CT([128, 4, 3], F32, "bqkv"); nc.sync.dma_start(t_bqkv[:], d_bqkv[:])
            t_isc = CT([128, C], BF16, "isc"); nc.sync.dma_start(t_isc[:], d_invsc[:])
            t_fc1b = CT([128, 16], F32, "fc1b"); nc.sync.dma_start(t_fc1b[:], d_fc1b[:])
            t_dwb = CT([128, 16], F32, "dwb"); nc.sync.dma_start(t_dwb[:], d_dwb[:])
            t_ln1g = CT([128, 16], F32, "ln1g"); nc.sync.dma_start(t_ln1g[:], d_ln1g[:])
            t_ln1b = CT([128, 16], F32, "ln1b"); nc.sync.dma_start(t_ln1b[:], d_ln1b[:])
            t_fc2b = CT([1, C], BF16, "fc2b"); nc.sync.dma_start(t_fc2b[:], d_fc2b[:])
            t_z128 = CT([128, 1], F32, "z128"); nc.vector.memset(t_z128[:], 0.0)
            t_e6 = CT([1, 1], F32, "e6"); nc.vector.memset(t_e6[:], 1e-6)
            t_eps = CT([1, 1], F32, "eps"); nc.vector.memset(t_eps[:], 1e-5)

            # ---- phase 1: QKV convs, k-chain, v bias ----
            t_x, t_w, t_kf, t_ev = [], [], [], []
            for i in range(4):
                xi = big.tile([128, N], BF16, tag=f"x{i}", name=f"x{i}")
                nc.sync.dma_start(xi[:], d_x[128 * i:128 * (i + 1), :])
                t_x.append(xi)
                wi = big.tile([128, 2048], BF16, tag=f"w{i}", name=f"w{i}")
                nc.sync.dma_start(wi[:, :3 * C], d_wqkvT[128 * i:128 * (i + 1), :])
                t_w.append(wi)
                ki = big.tile([128, N], BF16, tag=f"kf{i}", name=f"kf{i}")
                t_kf.append(ki)
                vi = big.tile([128, N], BF16, tag=f"ev{i}", name=f"ev{i}")
                t_ev.append(vi)

            def c512(tg):
                return chn.tile([128, 512], BF16, tag=tg, bufs=3, name=tg)

            def c1(tg):
                return chn.tile([128, 1], F32, tag=tg, bufs=3, name=tg)

            def focus(src_sl, dst_sl, prefix):
                """src (128,512) bf16 strictly-positive; writes focused output:
                v^3 * sqrt(sum(v^2)/sum(v^6)) into dst_sl."""
                vn2 = c1("n2")
                sq = c512("sq")
                nc.scalar.activation(sq[:], src_sl, AF.Square, bias=t_z128[:],
                                     accum_out=vn2[:])
                cube = c512("cube")
                nc.vector.tensor_mul(cube[:], src_sl, sq[:])
                v6 = c1("v6")
                junk = c512("sq")
                nc.scalar.activation(junk[:], cube[:], AF.Square, bias=t_z128[:],
                                     accum_out=v6[:])
                rv6 = c1("rv6")
                nc.vector.reciprocal(rv6[:], v6[:])
                rat = c1("rat")
                nc.vector.tensor_mul(rat[:], vn2[:], rv6[:])
                fac = c1("fac")
                nc.scalar.activation(fac[:], rat[:], AF.Sqrt, bias=t_z128[:])
                nc.vector.tensor_scalar_mul(dst_sl, cube[:], fac[:])

            for ob in range(4):
                for ch in range(8):
                    sl = slice(512 * ch, 512 * (ch + 1))
                    for m in range(3):
                        p = ps.tile([128, 512], F32, tag="ps", bufs=4, name="p_qkv")
                        for ct in range(4):
                            nc.tensor.matmul(
                                p[:],
                                t_w[ct][:, 512 * m + 128 * ob:512 * m + 128 * (ob + 1)],
                                t_x[ct][:, sl], start=(ct == 0), stop=(ct == 3))
                        bias = t_bqkv[:, ob, m:m + 1]
                        if m == 0:
                            eq = strm.tile([128, 512], BF16, tag="sA", bufs=4, name="eq")
                            nc.scalar.activation(eq[:], p[:], AF.Exp, bias=bias)
                            nc.sync.dma_start(d_eqs[128 * ob:128 * (ob + 1), sl], eq[:])
                        elif m == 2:
                            nc.scalar.activation(t_ev[ob][:, sl], p[:], AF.Identity,
                                                 bias=bias)
                        else:
                            esum = c1("esum")
                            ek = c512("ek")
                            nc.scalar.activation(ek[:], p[:], AF.Exp, bias=bias,
                                                 accum_out=esum[:])
                            r = c1("r")
                            nc.vector.reciprocal(r[:], esum[:])
                            k0 = c512("k0")
                            nc.vector.tensor_scalar(k0[:], ek[:], r[:], 1e-6,
                                                    op0=MUL, op1=ADD)
                            nc.vector.tensor_mul(k0[:], k0[:], t_isc[:])
                            focus(k0[:], t_kf[ob][:, sl], "k")

            if phases < 2: raise _Done
            # ---- phase 2: kv blocks + ksum ----
            t_kv = []
            for cb in range(4):
                p = ps.tile([128, 512], F32, tag="ps", bufs=4, name="p_kv")
                first = True
                for ob in range(4):
                    for g in range(8):
                        nc.tensor.matmul(
                            p[:],
                            t_kf[ob][:, 512 * g + 128 * cb:512 * g + 128 * (cb + 1)],
                            t_ev[ob][:, 512 * g:512 * (g + 1)],
                            start=first, stop=(ob == 3 and g == 7))
                        first = False
                kvc = CT([128, 512], BF16, f"kv{cb}")
                nc.vector.tensor_copy(kvc[:], p[:])
                t_kv.append(kvc)
            pk = ps.tile([1, 512], F32, tag="s1", bufs=1, name="p_ksum")
            first = True
            for ob in range(4):
                for g in range(8):
                    nc.tensor.matmul(pk[:], t_oc[:], t_kf[ob][:, 512 * g:512 * (g + 1)],
                                     start=first, stop=(ob == 3 and g == 7))
                    first = False
            ksb = Vb()
            nc.vector.tensor_copy(ksb[:], pk[:])
            nc.sync.dma_start(d_kss[:], ksb[:])
            t_ksC = CT([128, 4], BF16, "ksC")
            nc.sync.dma_start(t_ksC[:], d_kss.rearrange("o (jt p) -> p (o jt)", p=128))

            if phases < 3: raise _Done
            # ---- phase 3: q in A-layout ----
            t_eqA = []
            scr3 = d_eqs.rearrange("o (g j) -> o g j", g=8)
            for j in range(4):
                ea = big.tile([128, N], BF16, tag=f"kf{j}", name=f"eqA{j}")
                t_eqA.append(ea)
                for u in range(8):
                    t = 8 * j + u
                    nc.sync.dma_start(
                        ea[:, 512 * u:512 * (u + 1)],
                        scr3[16 * t:16 * (t + 1)].rearrange("o g j -> (o g) j"))
            dps = ps.tile([1, 512], F32, tag="s1", bufs=1, name="p_D")
            for t in range(32):
                nc.tensor.matmul(dps[:], t_oc[:],
                                 t_eqA[t // 8][:, 512 * (t % 8):512 * (t % 8 + 1)],
                                 start=(t == 0), stop=(t == 31))
            dv = V()
            nc.vector.tensor_copy(dv[:], dps[:])
            rd = V()
            nc.vector.reciprocal(rd[:], dv[:])
            cd = V()
            nc.vector.tensor_mul(cd[:], rd[:], t_isc[0:1, :])
            cdb = Vb()
            nc.vector.tensor_copy(cdb[:], cd[:])
            bp = ps.tile([128, 512], F32, tag="bc", bufs=1, name="p_bc1")
            nc.tensor.matmul(bp[:], t_or[:], cdb[:], start=True, stop=True)
            cd_bc = CT([128, 512], BF16, "cdbc")
            nc.vector.tensor_copy(cd_bc[:], bp[:])

            t_qfT = []
            for j in range(4):
                qt = big.tile([128, N], BF16, tag=f"x{j}", name=f"qfT{j}")
                t_qfT.append(qt)
            for t in range(32):
                src = t_eqA[t // 8][:, 512 * (t % 8):512 * (t % 8 + 1)]
                q0 = c512("ek")
                nc.vector.tensor_mul(q0[:], src, cd_bc[:])
                nc.vector.tensor_scalar_add(q0[:], q0[:], 1e-6)
                qf = c512("k0")
                focus(q0[:], qf[:], "q")
                for jt in range(4):
                    nc.sync.dma_start_transpose(
                        t_qfT[jt][:, 128 * t:128 * (t + 1)],
                        qf[:, 128 * jt:128 * (jt + 1)])

            if phases < 4: raise _Done
            # ---- phase 4: z; fold into qfT (in-place -> qzT) ----
            for ch in range(8):
                sl = slice(512 * ch, 512 * (ch + 1))
                zp = ps.tile([1, 512], F32, tag="s1", bufs=1, name="p_z")
                for jt in range(4):
                    nc.tensor.matmul(zp[:], t_ksC[:, jt:jt + 1], t_qfT[jt][:, sl],
                                     start=(jt == 0), stop=(jt == 3))
                za = V()
                nc.scalar.activation(za[:], zp[:], AF.Identity, bias=t_e6[:])
                zr = V()
                nc.vector.reciprocal(zr[:], za[:])
                zrb = Vb()
                nc.vector.tensor_copy(zrb[:], zr[:])
                zbp = ps.tile([128, 512], F32, tag="bc", bufs=1, name="p_bc2")
                nc.tensor.matmul(zbp[:], t_or[:], zrb[:], start=True, stop=True)
                zbc = strm.tile([128, 512], BF16, tag="sA", bufs=4, name="zbc")
                nc.vector.tensor_copy(zbc[:], zbp[:])
                for jt in range(4):
                    nc.vector.tensor_mul(t_qfT[jt][:, sl], t_qfT[jt][:, sl], zbc[:])

            if phases < 5: raise _Done
            # ---- phase 5: attnT -> enhT -> lnm -> fc1 -> a_scr ----
            t_fc1 = []
            for j in range(4):
                f1 = big.tile([128, 2048], BF16, tag=f"w{j}", name=f"fc1_{j}")
                nc.sync.dma_start(f1[:], d_fc1[128 * j:128 * (j + 1), :])
                t_fc1.append(f1)

            def ln_stats_bcast(sps, s2ps, nfeat):
                mu, m2, var, mrs = V(), V(), V(), V()
                nc.scalar.mul(mu[:], sps[:], 1.0 / nfeat)
                nc.scalar.mul(m2[:], s2ps[:], 1.0 / nfeat)
                nc.vector.tensor_mul(var[:], mu[:], mu[:])
                nc.vector.tensor_sub(var[:], m2[:], var[:])
                sd = V()
                nc.scalar.activation(sd[:], var[:], AF.Sqrt, bias=t_eps[:])
                rstd = V()
                nc.vector.reciprocal(rstd[:], sd[:])
                nc.vector.tensor_mul(mrs[:], mu[:], rstd[:])
                rstdb, mrsb = Vb(), Vb()
                nc.vector.tensor_copy(rstdb[:], rstd[:])
                nc.vector.tensor_copy(mrsb[:], mrs[:])
                out = []
                for vsrc, tg in ((rstdb, "rstd_bc"), (mrsb, "mrs_bc")):
                    bp = ps.tile([128, 512], F32, tag="bc", bufs=1, name="p_bc3")
                    nc.tensor.matmul(bp[:], t_or[:], vsrc[:], start=True, stop=True)
                    ob = strm.tile([128, 512], BF16, tag=tg, bufs=2, name=tg)
                    nc.vector.tensor_copy(ob[:], bp[:])
                    out.append(ob)
                return out

            for ch in range(8):
                sl = slice(512 * ch, 512 * (ch + 1))
                enh, esq, xch = [], [], []
                for db in range(4):
                    xc = strm.tile([128, 512], BF16, tag=f"xch{db}", bufs=2,
                                   name=f"xch{db}")
                    nc.sync.dma_start(xc[:], d_x[128 * db:128 * (db + 1), sl])
                    ap = ps.tile([128, 512], F32, tag="ps", bufs=4, name="p_at")
                    for jt in range(4):
                        nc.tensor.matmul(ap[:], t_kv[jt][:, 128 * db:128 * (db + 1)],
                                         t_qfT[jt][:, sl], start=(jt == 0),
                                         stop=(jt == 3))
                    en = strm.tile([128, 512], BF16, tag=f"enh{db}", bufs=2,
                                   name=f"enh{db}")
                    nc.vector.tensor_add(en[:], ap[:], xc[:])
                    es = strm.tile([128, 512], BF16, tag="sA", bufs=4, name="esq")
                    nc.scalar.activation(es[:], en[:], AF.Square, bias=t_z128[:])
                    enh.append(en)
                    esq.append(es)
                sps = ps.tile([1, 512], F32, tag="s1", bufs=1, name="p_s1")
                s2ps = ps.tile([1, 512], F32, tag="s2", bufs=1, name="p_s2")
                for db in range(4):
                    nc.tensor.matmul(sps[:], t_oc[:], enh[db][:],
                                     start=(db == 0), stop=(db == 3))
                    nc.tensor.matmul(s2ps[:], t_oc[:], esq[db][:],
                                     start=(db == 0), stop=(db == 3))
                rstd_bc, mrs_bc = ln_stats_bcast(sps, s2ps, C)
                thT = []
                for db in range(4):
                    th = strm.tile([128, 512], BF16, tag=f"thT{db}", bufs=2,
                                   name=f"thT{db}")
                    nc.vector.tensor_mul(th[:], enh[db][:], rstd_bc[:])
                    nc.vector.tensor_sub(th[:], th[:], mrs_bc[:])
                    thT.append(th)
                for i in range(16):
                    ap = ps.tile([128, 512], F32, tag="ps", bufs=4, name="p_fc1")
                    for db in range(4):
                        nc.tensor.matmul(ap[:], t_fc1[db][:, 128 * i:128 * (i + 1)],
                                         thT[db][:], start=(db == 0), stop=(db == 3))
                    ae = strm.tile([128, 512], BF16, tag="sA", bufs=4, name="ae")
                    nc.scalar.activation(ae[:], ap[:], AF.Identity,
                                         bias=t_fc1b[:, i:i + 1])
                    nc.sync.dma_start(d_ascr[i, :, sl], ae[:])

            if phases < 6: raise _Done
            # ---- phase 6a: depthwise conv -> y_scr ----
            a3 = d_ascr.rearrange("i p (h w) -> i p h w", h=H)
            taps = [(0, 0)] + [(dy, dx) for dy in (-1, 0, 1)
                               for dx in (-1, 0, 1) if (dy, dx) != (0, 0)]
            for i in range(16):
                dd = strm.tile([128, 9, 128], BF16, tag="dd", bufs=2, name="dd")
                nc.sync.dma_start(dd[:], d_dwd[i].rearrange("p (t m) -> p t m", t=9))
                for ch in range(8):
                    h0 = 8 * ch
                    r0, r1 = max(0, h0 - 1), min(H, h0 + 9)
                    aw = strm.tile([128, 10, 64], BF16, tag="aw", bufs=3, name="aw")
                    nc.sync.dma_start(aw[:, :r1 - r0], a3[i, :, r0:r1, :])
                    dp = ps.tile([128, 8, 64], F32, tag="ps", bufs=4, name="p_dw")
                    for ti, (dy, dx) in enumerate(taps):
                        lo = 1 if (h0 + dy) < 0 else 0
                        hi = 7 if (h0 + 7 + dy) > (H - 1) else 8
                        rows = hi - lo
                        src0 = h0 + lo + dy - r0
                        wlo = max(0, -dx)
                        wn = 64 - abs(dx)
                        t = (dy + 1) * 3 + (dx + 1)
                        nc.tensor.matmul(
                            dp[:, lo:lo + rows, wlo:wlo + wn],
                            dd[:, t],
                            aw[:, src0:src0 + rows, wlo + dx:wlo + dx + wn],
                            start=(ti == 0), stop=(ti == 9))
                    ye = strm.tile([128, 512], BF16, tag="sA", bufs=4, name="ye")
                    nc.scalar.activation(ye[:], dp.rearrange("p h w -> p (h w)"),
                                         AF.Identity, bias=t_dwb[:, i:i + 1])
                    nc.sync.dma_start(d_yscr[i, :, 512 * ch:512 * (ch + 1)], ye[:])

            if phases < 7: raise _Done
            # ---- phase 6b/7: ln1 + gelu + fc2 + residuals -> out ----
            t_fc2 = []
            for j in range(4):
                f2 = big.tile([128, 4, 512], BF16, tag=f"kf{j}", name=f"fc2_{j}")
                nc.sync.dma_start(
                    f2[:], d_fc2[512 * j:512 * (j + 1), :].rearrange(
                        "(m p) c -> p m c", p=128))
                t_fc2.append(f2)
            for ch in range(8):
                sl = slice(512 * ch, 512 * (ch + 1))
                t_y = []
                for j in range(4):
                    yb = big.tile([128, 4, 512], BF16, tag=f"ev{j}", name=f"y{j}")
                    nc.sync.dma_start(
                        yb[:], d_yscr[4 * j:4 * (j + 1), :, sl].rearrange(
                            "m p c -> p m c"))
                    t_y.append(yb)
                sps = ps.tile([1, 512], F32, tag="s1", bufs=1, name="p_s1b")
                s2ps = ps.tile([1, 512], F32, tag="s2", bufs=1, name="p_s2b")
                for i in range(16):
                    ysl = t_y[i // 4][:, i % 4, :]
                    ysq = strm.tile([128, 512], BF16, tag="sA", bufs=4, name="ysq")
                    nc.scalar.activation(ysq[:], ysl, AF.Square, bias=t_z128[:])
                    nc.tensor.matmul(sps[:], t_oc[:], ysl,
                                     start=(i == 0), stop=(i == 15))
                    nc.tensor.matmul(s2ps[:], t_oc[:], ysq[:],
                                     start=(i == 0), stop=(i == 15))
                rstd_bc, mrs_bc = ln_stats_bcast(sps, s2ps, HID)
                t_g = []
                for j in range(4):
                    gb = big.tile([128, 4, 512], BF16, tag=f"w{j}", name=f"g{j}")
                    t_g.append(gb)
                for i in range(16):
                    ysl = t_y[i // 4][:, i % 4, :]
                    nc.vector.tensor_mul(ysl, ysl, rstd_bc[:])
                    nc.vector.tensor_sub(ysl, ysl, mrs_bc[:])
                    nc.scalar.activation(t_g[i // 4][:, i % 4, :], ysl, AF.Gelu,
                                         bias=t_ln1b[:, i:i + 1],
                                         scale=t_ln1g[:, i:i + 1])
                for s in range(4):
                    nsl = slice(512 * ch + 128 * s, 512 * ch + 128 * (s + 1))
                    xt = strm.tile([128, 512], BF16, tag="xt", bufs=3, name="xt")
                    nc.sync.dma_start(xt[:], d_xT[nsl, :])
                    op = ps.tile([128, 512], F32, tag="ps", bufs=4, name="p_out")
                    for i in range(16):
                        nc.tensor.matmul(op[:],
                                         t_g[i // 4][:, i % 4, 128 * s:128 * (s + 1)],
                                         t_fc2[i // 4][:, i % 4, :],
                                         start=(i == 0), stop=False)
                    for jt in range(4):
                        nc.tensor.matmul(op[:], t_qfT[jt][:, nsl], t_kv[jt][:],
                                         start=False, stop=False)
                    nc.tensor.matmul(op[:], t_eye[:], xt[:], start=False, stop=False)
                    nc.tensor.matmul(op[:], t_or[:], t_fc2b[:], start=False, stop=True)
                    ob = strm.tile([128, 512], BF16, tag="ob", bufs=3, name="ob")
                    nc.vector.tensor_copy(ob[:], op[:])
                    nc.sync.dma_start(d_out[nsl, :], ob[:])

          except _Done:
            pass
    nc.compile()
    return nc


def _prep_shared(inp):
    bf = ml_dtypes.bfloat16
    f32 = np.float32
    Wq, Wk, Wv = [np.asarray(inp[k], f32) for k in ("Wq", "Wk", "Wv")]
    wqkvT = np.ascontiguousarray(
        np.concatenate([Wq.T, Wk.T, Wv.T], axis=1)).astype(bf)
    bqkv = np.ascontiguousarray(np.stack(
        [np.asarray(inp[k], f32).reshape(4, 128).T for k in ("bq", "bk", "bv")],
        axis=2))                                     # (128, 4, 3)
    sc = np.log1p(np.exp(np.asarray(inp["scale"], f32))).reshape(C)
    invsc_bc = np.ascontiguousarray(
        np.broadcast_to((1.0 / sc)[None, :], (128, C))).astype(bf)
    lnm_g = np.asarray(inp["lnm_g"], f32)
    lnm_b = np.asarray(inp["lnm_b"], f32)
    fc1w = np.asarray(inp["fc1_w"], f32)
    fc1p = (lnm_g[:, None] * fc1w).astype(bf)
    fc1bp = np.asarray(inp["fc1_b"], f32) + lnm_b @ fc1w
    fc1bp = np.ascontiguousarray(fc1bp.reshape(16, 128).T)      # (128, 16)
    fc2w = np.asarray(inp["fc2_w"], f32).astype(bf)
    fc2b = np.asarray(inp["fc2_b"], f32).astype(bf).reshape(1, C)
    dww = np.asarray(inp["dw_w"], f32).reshape(HID, 9).copy()
    dww[:, 4] += 1.0                     # fold the "+a" residual into center tap
    dwdiag = np.zeros((16, 128, 9, 128), f32)
    ii = np.arange(128)
    for i in range(16):
        for t in range(9):
            dwdiag[i, ii, t, ii] = dww[128 * i + ii, t]
    dwdiag = dwdiag.reshape(16, 128, 9 * 128).astype(bf)
    dwb = np.ascontiguousarray(np.asarray(inp["dw_b"], f32).reshape(16, 128).T)
    ln1g = np.ascontiguousarray(np.asarray(inp["ln1_g"], f32).reshape(16, 128).T)
    ln1b = np.ascontiguousarray(np.asarray(inp["ln1_b"], f32).reshape(16, 128).T)
    return {
        "wqkvT": wqkvT, "bqkv": bqkv, "invsc_bc": invsc_bc,
        "fc1p": fc1p, "fc1bp": fc1bp, "fc2w": fc2w, "fc2b": fc2b,
        "dwdiag": dwdiag, "dwb": dwb, "ln1g": ln1g, "ln1b": ln1b,
        "eye": np.eye(128, dtype=f32).astype(bf),
        "onesc": np.ones((128, 1), f32).astype(bf),
        "onesr": np.ones((1, 128), f32).astype(bf),
    }


def run(inputs, trace=False, **kw):
    from concourse.bass_utils import run_bass_kernel_spmd
    bf = ml_dtypes.bfloat16
    if "nc" not in _CACHE:
        _CACHE["nc"] = _build()
    nc = _CACHE["nc"]
    shared = _prep_shared(inputs)
    x = np.asarray(inputs["x"], np.float32).reshape(B, C, N)
    in_maps = []
    for b in range(B):
        m = dict(shared)
        m["x"] = x[b].astype(bf)
        m["xT"] = np.ascontiguousarray(x[b].T).astype(bf)
        in_maps.append(m)
    res = run_bass_kernel_spmd(nc, in_maps, list(range(NCORES)), trace=trace, **kw)
    outs = [np.asarray(res.results[b]["out"], np.float32).reshape(H, W, C)
            for b in range(B)]
    return np.stack(outs), res


def kernel(**inputs):
    out, _ = run(inputs, trace=False)
    return out
